# revision 8
# baseline (speedup 1.0000x reference)
"""PhysicsAttentionV3 Trainium2 kernel (8-core SPMD).

Strategy: shard N (mesh points) across 8 cores. Per core, stream 128-row
tiles: slice-logits matmul (fp16), softmax over M per head, accumulate
s_raw (w^T x) and d (colsum w) in PSUM across all tiles, AllReduce the
(H*M, C) / (H*M,) accumulators, run the tiny MxM attention replicated,
then deslice (w @ s_out) per tile from an SBUF-resident fp16 w^T stash.
"""

import numpy as np

import concourse.bass as bass
import concourse.mybir as mybir
import concourse.tile as tile
from concourse import bacc
from concourse.bass import ts
from concourse.bass_utils import run_bass_kernel_spmd
from concourse.masks import make_identity

F32 = mybir.dt.float32
F16 = mybir.dt.float16

B, N, C = 1, 100_000, 512
H, DH, M = 8, 64, 64
E = H * M  # 512
NCORES = 8
RPC = 12544          # padded rows per core (98 * 128)
T = RPC // 128       # 98 tiles
TAIL_REAL = N // NCORES - (T - 1) * 128  # 84 real rows in last tile


def build(with_bslice: bool, debug: bool = False):
    nc = bacc.Bacc("TRN2", target_bir_lowering=False, debug=False,
                   num_devices=NCORES)

    xh = nc.declare_dram_parameter("xh", [RPC, C], F16, isOutput=False)
    wst = nc.declare_dram_parameter("wst", [C, E], F16, isOutput=False)
    w1t = nc.declare_dram_parameter("w1t", [C, DH], F32, isOutput=False)
    b1 = nc.declare_dram_parameter("b1", [1, DH], F32, isOutput=False)
    wqt = nc.declare_dram_parameter("wqt", [DH, DH], F32, isOutput=False)
    wkt = nc.declare_dram_parameter("wkt", [DH, DH], F32, isOutput=False)
    wvt = nc.declare_dram_parameter("wvt", [DH, DH], F32, isOutput=False)
    w3t = nc.declare_dram_parameter("w3t", [DH, DH], F32, isOutput=False)
    b3 = nc.declare_dram_parameter("b3", [1, DH], F32, isOutput=False)
    tmask = nc.declare_dram_parameter("tmask", [128, 1], F32, isOutput=False)
    if with_bslice:
        bse = nc.declare_dram_parameter("bse", [1, E], F16, isOutput=False)
    out = nc.declare_dram_parameter("out", [RPC, C], F32, isOutput=True)
    if debug:
        dbg_ww = nc.declare_dram_parameter("dbg_ww", [128, E], F16, isOutput=True)
        dbg_xt = nc.declare_dram_parameter("dbg_xt", [128, 4 * 512], F16, isOutput=True)
        dbg_wt = nc.declare_dram_parameter("dbg_wt", [128, 4 * 128], F16, isOutput=True)
        dbg_d2 = nc.declare_dram_parameter("dbg_d2", [128, 4], F32, isOutput=True)
        dbg_sr = nc.declare_dram_parameter("dbg_sr", [128, 4 * C], F32, isOutput=True)
        dbg_s = nc.declare_dram_parameter("dbg_s", [128, 4 * DH], F32, isOutput=True)
        dbg_bd = nc.declare_dram_parameter("dbg_bd", [128, 4 * 128], F16, isOutput=True)

    with tile.TileContext(nc) as tc:
        with (
            tc.tile_pool(name="const", bufs=1) as const,
            tc.tile_pool(name="stash", bufs=1) as stash,
            tc.tile_pool(name="sb", bufs=3) as sb,
            tc.tile_pool(name="xtp", bufs=2) as xtp,
            tc.tile_pool(name="pacc", bufs=1, space="PSUM") as pacc,
            tc.tile_pool(name="pwork", bufs=3, space="PSUM") as pwork,
            tc.tile_pool(name="dram", bufs=1, space="DRAM") as dram,
        ):
            # ---- constants ----
            wst_sb = const.tile([128, 4, E], F16)
            nc.sync.dma_start(wst_sb[:], wst.rearrange("(q p) e -> p q e", p=128))
            w1t_sb = const.tile([128, 4, DH], F32)
            nc.sync.dma_start(w1t_sb[:], w1t.rearrange("(q p) d -> p q d", p=128))
            b1_sb = const.tile([1, DH], F32)
            nc.sync.dma_start(b1_sb[:], b1[:])
            b3_sb = const.tile([1, DH], F32)
            nc.sync.dma_start(b3_sb[:], b3[:])
            small = {}
            for name, t_ in (("wqt", wqt), ("wkt", wkt), ("wvt", wvt), ("w3t", w3t)):
                s_ = const.tile([DH, DH], F32, tag=name, name=name + "_sb")
                nc.sync.dma_start(s_[:], t_[:])
                small[name] = s_
            if with_bslice:
                bse_sb = const.tile([1, E], F16)
                nc.sync.dma_start(bse_sb[:], bse[:])
            tmask_sb = const.tile([128, 1], F32)
            nc.sync.dma_start(tmask_sb[:], tmask[:])
            ones16 = const.tile([128, 1], F16)
            nc.vector.memset(ones16[:], 1.0)
            ones16r = const.tile([1, 128], F16)
            nc.vector.memset(ones16r[:], 1.0)
            ones32r = const.tile([1, 128], F32)
            nc.vector.memset(ones32r[:], 1.0)
            ident = const.tile([128, 128], F32)
            make_identity(nc, ident[:])

            # persistent w^T stash: [e-in-chunk, tile, chunk, n-in-tile] fp16
            wt_stash = stash.tile([128, T, 4, 128], F16)

            # PSUM accumulators (accumulate across all T tiles)
            ps_sr = [pacc.tile([128, C], F32, tag=f"sr{j}", name=f"ps_sr{j}") for j in range(4)]
            ps_d = pacc.tile([1, E], F32, tag="d")

            # ================= Phase A: per-tile streaming =================
            for t in range(T):
                r0 = t * 128
                g4 = t % 4
                if g4 == 0:
                    # x^T for 4 tiles: per c-chunk q, [512 rows, 128 cols] -> [128, 512]
                    xt4 = xtp.tile([128, 4, 512], F16, tag="xt4")
                    nrows = min(512, RPC - r0)
                    for q in range(4):
                        nc.sync.dma_start(
                            xt4[:, q, :nrows],
                            xh[r0:r0 + nrows, ts(q, 128)],
                            transpose=True,
                        )
                xn = sb.tile([128, C], F16, tag="xn")
                nc.sync.dma_start(xn[:], xh[r0:r0 + 128, :])

                # logits: psum [n, e] = sum_q xT_q.T @ wstT_q
                psl = pwork.tile([128, E], F32, tag="pwork")
                for q in range(4):
                    nc.tensor.matmul(
                        psl[:], xt4[:, q, ts(g4, 128)], wst_sb[:, q, :],
                        start=(q == 0), stop=(q == 3 and not with_bslice),
                    )
                if with_bslice:
                    nc.tensor.matmul(psl[:], ones16r[:], bse_sb[:],
                                     start=False, stop=True)

                # u = exp(logits); z = per-head rowsum; w = u / z   (fp16)
                u = sb.tile([128, E], F32, tag="u")
                nc.scalar.activation(u[:], psl[:], mybir.ActivationFunctionType.Exp)
                z = sb.tile([128, H], F32, tag="z")
                nc.vector.reduce_sum(
                    z[:], u[:].rearrange("p (h m) -> p h m", h=H),
                    axis=mybir.AxisListType.X,
                )
                zi = sb.tile([128, H], F32, tag="zi")
                nc.vector.reciprocal(zi[:], z[:])
                if t == T - 1:
                    nc.vector.tensor_scalar_mul(zi[:], zi[:], tmask_sb[:])
                ww = sb.tile([128, E], F16, tag="ww")
                nc.vector.tensor_mul(
                    ww[:].rearrange("p (h m) -> p h m", h=H),
                    u[:].rearrange("p (h m) -> p h m", h=H),
                    zi[:, :, None].broadcast_to([128, H, M]),
                )

                # s_raw[e,c] += w_chunk.T @ x ; d[e] += ones.T @ w
                for j in range(4):
                    nc.tensor.matmul(ps_sr[j][:], ww[:, ts(j, 128)], xn[:],
                                     start=(t == 0), stop=(t == T - 1))
                nc.tensor.matmul(ps_d[:], ones16[:], ww[:],
                                 start=(t == 0), stop=(t == T - 1))

                if debug and t == 0:
                    nc.sync.dma_start(dbg_ww[:], ww[:])
                    nc.sync.dma_start(dbg_xt[:], xt4[:].rearrange("p q f -> p (q f)"))
                # stash w^T (SBUF->SBUF DMA transpose)
                for j in range(4):
                    nc.sync.dma_start(wt_stash[:, t, j, :], ww[:, ts(j, 128)],
                                      transpose=True)

            if debug:
                nc.sync.dma_start(dbg_wt[:], wt_stash[:, 0].rearrange("p j n -> p (j n)"))
            # ================= Phase B: AllReduce =================
            sr_sb = sb.tile([128, 4, C], F32, tag="sr_sb", bufs=1)
            for j in range(4):
                nc.scalar.copy(sr_sb[:, j, :], ps_sr[j][:])
            d_sb = sb.tile([1, E], F32, tag="d_sb", bufs=1)
            nc.scalar.copy(d_sb[:], ps_d[:])

            cc_in = dram.tile([129, 2048], F32)
            cc_out = dram.tile([129, 2048], F32, addr_space="Shared")
            nc.sync.dma_start(cc_in[0:128, :], sr_sb[:].rearrange("p j c -> p (j c)"))
            nc.sync.dma_start(cc_in[128:129, 0:E], d_sb[:])
            nc.gpsimd.collective_compute(
                "AllReduce", mybir.AluOpType.add,
                replica_groups=[list(range(NCORES))],
                ins=[cc_in.opt()], outs=[cc_out.opt()],
            )
            sr2 = sb.tile([128, 4, C], F32, tag="sr2", bufs=1)
            nc.sync.dma_start(sr2[:].rearrange("p j c -> p (j c)"), cc_out[0:128, :])
            d2 = sb.tile([128, 4], F32, tag="d2", bufs=1)
            nc.sync.dma_start(d2[:], cc_out[128, 0:E].rearrange("(j p) -> p j", p=128))

            if debug:
                nc.sync.dma_start(dbg_d2[:], d2[:])
                nc.sync.dma_start(dbg_sr[:], sr2[:].rearrange("p j c -> p (j c)"))
            # ================= Phase C: tiny attention (replicated) ========
            dd = sb.tile([128, 4], F32, tag="dd", bufs=1)
            nc.vector.tensor_scalar_add(dd[:], d2[:], 1e-5)
            nc.vector.reciprocal(dd[:], dd[:])
            sn = sb.tile([128, 4, C], F32, tag="sn", bufs=1)
            for j in range(4):
                nc.vector.tensor_scalar_mul(sn[:, j, :], sr2[:, j, :], dd[:, j:j + 1])

            # sn^T: [c-chunk q partitions, e] fp32
            snt = sb.tile([128, 4, E], F32, tag="snt", bufs=1)
            for q in range(4):
                pst = pwork.tile([128, E], F32, tag="pwork")
                for j in range(4):
                    nc.tensor.transpose(pst[:, ts(j, 128)], sn[:, j, ts(q, 128)],
                                        ident[:])
                nc.scalar.copy(snt[:, q, :], pst[:])

            # s[e, dh] = sn @ w1^T + b1
            s_sb = sb.tile([128, 4, DH], F32, tag="s_sb", bufs=1)
            for g in range(4):
                pss = pwork.tile([128, DH], F32, tag="pwork")
                for q in range(4):
                    nc.tensor.matmul(pss[:], snt[:, q, ts(g, 128)], w1t_sb[:, q, :],
                                     start=(q == 0), stop=False)
                nc.tensor.matmul(pss[:], ones32r[:], b1_sb[:], start=False, stop=True)
                nc.scalar.copy(s_sb[:, g, :], pss[:])

            # per-head attention; bd[g] = blockdiag(s_out_{2g}, s_out_{2g+1}) fp16
            bd = [const.tile([128, 128], F16, tag=f"bd{g}", name=f"bd{g}") for g in range(4)]
            for g in range(4):
                nc.vector.memset(bd[g][:], 0.0)
            for h in range(H):
                g, half = h // 2, (h % 2) * 64
                s_h = s_sb[half:half + 64, g, :]
                pt = pwork.tile([64, 64], F32, tag="pwork")
                nc.tensor.transpose(pt[:], s_h, ident[half:half + 64, half:half + 64])
                st_h = sb.tile([64, 64], F32, tag="st_h")
                nc.scalar.copy(st_h[:], pt[:])

                pq = pwork.tile([64, 64], F32, tag="pwork")
                nc.tensor.matmul(pq[:], small["wqt"][:], st_h[:])
                qt_h = sb.tile([64, 64], F32, tag="qt_h")
                nc.scalar.copy(qt_h[:], pq[:])
                pk = pwork.tile([64, 64], F32, tag="pwork")
                nc.tensor.matmul(pk[:], small["wkt"][:], st_h[:])
                kt_h = sb.tile([64, 64], F32, tag="kt_h")
                nc.scalar.copy(kt_h[:], pk[:])
                pv = pwork.tile([64, 64], F32, tag="pwork")
                nc.tensor.matmul(pv[:], st_h[:], small["wvt"][:])
                v_h = sb.tile([64, 64], F32, tag="v_h")
                nc.scalar.copy(v_h[:], pv[:])

                pA = pwork.tile([64, 64], F32, tag="pwork")
                nc.tensor.matmul(pA[:], qt_h[:], kt_h[:])
                mx = sb.tile([64, 1], F32, tag="mx")
                nc.vector.reduce_max(mx[:], pA[:], axis=mybir.AxisListType.X)
                nmx = sb.tile([64, 1], F32, tag="nmx")
                nc.vector.tensor_scalar_mul(nmx[:], mx[:], -1.0)
                p_sb = sb.tile([64, 64], F32, tag="p_sb")
                nc.scalar.activation(p_sb[:], pA[:],
                                     mybir.ActivationFunctionType.Exp, bias=nmx[:])
                rs = sb.tile([64, 1], F32, tag="rs")
                nc.vector.reduce_sum(rs[:], p_sb[:], axis=mybir.AxisListType.X)
                ri = sb.tile([64, 1], F32, tag="ri")
                nc.vector.reciprocal(ri[:], rs[:])
                pn = sb.tile([64, 64], F32, tag="pn")
                nc.vector.tensor_scalar_mul(pn[:], p_sb[:], ri[:])

                pat = pwork.tile([64, 64], F32, tag="pwork")
                nc.tensor.transpose(pat[:], pn[:], ident[0:64, 0:64])
                at_h = sb.tile([64, 64], F32, tag="at_h")
                nc.scalar.copy(at_h[:], pat[:])

                psat = pwork.tile([64, 64], F32, tag="pwork")
                nc.tensor.matmul(psat[:], v_h[:], at_h[:])
                sat_h = sb.tile([64, 64], F32, tag="sat_h")
                nc.scalar.copy(sat_h[:], psat[:])

                pso = pwork.tile([128, 64], F32, tag="pwork")
                nc.tensor.matmul(pso[half:half + 64, :], sat_h[:], small["w3t"][:],
                                 start=True, stop=False)
                nc.tensor.matmul(pso[half:half + 64, :], ones32r[:, 0:64], b3_sb[:],
                                 start=False, stop=True)
                nc.scalar.copy(bd[g][half:half + 64, half:half + 64],
                               pso[half:half + 64, :])

            if debug:
                nc.sync.dma_start(dbg_s[:], s_sb[:].rearrange("p j d -> p (j d)"))
                for g in range(4):
                    nc.sync.dma_start(dbg_bd[:, g * 128:(g + 1) * 128], bd[g][:])
            # ================= Phase D: deslice =================
            for t in range(T):
                r0 = t * 128
                po = pwork.tile([128, C], F32, tag="pwork")
                for g in range(4):
                    nc.tensor.matmul(po[:, ts(g, 128)], wt_stash[:, t, g, :],
                                     bd[g][:], start=True, stop=True)
                osb = sb.tile([128, C], F32, tag="osb")
                nc.scalar.copy(osb[:], po[:])
                nc.sync.dma_start(out[r0:r0 + 128, :], osb[:])

    nc.finalize()
    return nc


_CACHE = {}


TAIL = TAIL_REAL


def kernel(x, temperature, w_slice, b_slice, w1, b1, wq, wk, wv, w3, b3):
    x = np.asarray(x, np.float32)
    temperature = np.asarray(temperature, np.float32)
    w_slice = np.asarray(w_slice, np.float32)
    b_slice = np.asarray(b_slice, np.float32)
    w1 = np.asarray(w1, np.float32)
    b1 = np.asarray(b1, np.float32)
    wq = np.asarray(wq, np.float32)
    wk = np.asarray(wk, np.float32)
    wv = np.asarray(wv, np.float32)
    w3 = np.asarray(w3, np.float32)
    b3 = np.asarray(b3, np.float32)

    temp = np.clip(temperature.reshape(H), 0.1, 5.0)
    se = np.repeat(1.0 / temp, M)                      # per-e scale
    weff_t = (w_slice * se[:, None]).T                 # [C, E]
    beff = b_slice * se
    with_bslice = bool(np.any(beff != 0.0))

    key = with_bslice
    if key not in _CACHE:
        _CACHE[key] = build(with_bslice)
    nc = _CACHE[key]

    xp = np.zeros((NCORES, RPC, C), np.float16)
    rows = N // NCORES
    xs = x.reshape(N, C)
    for i in range(NCORES):
        xp[i, :rows] = xs[i * rows:(i + 1) * rows]

    base = {
        "wst": weff_t.astype(np.float16),
        "w1t": np.ascontiguousarray(w1.T),
        "b1": b1.reshape(1, DH),
        "wqt": np.ascontiguousarray(wq.T) * np.float32(1.0 / np.sqrt(DH)),
        "wkt": np.ascontiguousarray(wk.T),
        "wvt": np.ascontiguousarray(wv.T),
        "w3t": np.ascontiguousarray(w3.T),
        "b3": b3.reshape(1, DH),
        "tmask": (np.arange(128) < TAIL_REAL).astype(np.float32).reshape(128, 1),
    }
    if with_bslice:
        base["bse"] = beff.reshape(1, E).astype(np.float16)
    in_maps = [{"xh": xp[i], **base} for i in range(NCORES)]

    res = run_bass_kernel_spmd(nc, in_maps, list(range(NCORES)))
    rows = N // NCORES
    full = np.concatenate([res.results[i]["out"][:rows] for i in range(NCORES)], axis=0)
    return full.reshape(B, N, C)


# revision 28
# speedup vs baseline: 1.0286x; 1.0286x over previous
"""PhysicsAttentionV3 Trainium2 kernel (8-core SPMD).

Strategy: shard N (mesh points) across 8 cores. Per core, stream 128-row
tiles: slice-logits matmul (fp16), softmax over M per head, accumulate
s_raw (w^T x) and d (colsum w) in PSUM across all tiles, AllReduce the
(H*M, C) / (H*M,) accumulators, run the tiny MxM attention replicated,
then deslice (w @ s_out) per tile from an SBUF-resident fp16 w^T stash.
"""

import numpy as np

import concourse.bass as bass
import concourse.mybir as mybir
import concourse.tile as tile
from concourse import bacc
from concourse.bass import ts
from concourse.bass_utils import run_bass_kernel_spmd
from concourse.masks import make_identity

F32 = mybir.dt.float32
F16 = mybir.dt.float16

B, N, C = 1, 100_000, 512
H, DH, M = 8, 64, 64
E = H * M  # 512
NCORES = 8
RPC = 12544          # padded rows per core (98 * 128)
T = RPC // 128       # 98 tiles
TAIL_REAL = N // NCORES - (T - 1) * 128  # 84 real rows in last tile


def build(with_bslice: bool, debug: bool = False, single: bool = False):
    nc = bacc.Bacc("TRN2", target_bir_lowering=False, debug=False,
                   num_devices=1 if single else NCORES)

    xh = nc.declare_dram_parameter("xh", [RPC, C], F16, isOutput=False)
    wst = nc.declare_dram_parameter("wst", [C, E], F16, isOutput=False)
    w1t = nc.declare_dram_parameter("w1t", [C, DH], F32, isOutput=False)
    b1 = nc.declare_dram_parameter("b1", [1, DH], F32, isOutput=False)
    wqt = nc.declare_dram_parameter("wqt", [DH, DH], F32, isOutput=False)
    wkt = nc.declare_dram_parameter("wkt", [DH, DH], F32, isOutput=False)
    wvt = nc.declare_dram_parameter("wvt", [DH, DH], F32, isOutput=False)
    w3t = nc.declare_dram_parameter("w3t", [DH, DH], F32, isOutput=False)
    b3 = nc.declare_dram_parameter("b3", [1, DH], F32, isOutput=False)
    tmask = nc.declare_dram_parameter("tmask", [128, 1], F32, isOutput=False)
    if with_bslice:
        bse = nc.declare_dram_parameter("bse", [1, E], F16, isOutput=False)
    out = nc.declare_dram_parameter("out", [RPC, C], F16, isOutput=True)
    if debug:
        dbg_ww = nc.declare_dram_parameter("dbg_ww", [128, E], F16, isOutput=True)
        dbg_xt = nc.declare_dram_parameter("dbg_xt", [128, 4 * 512], F16, isOutput=True)
        dbg_wt = nc.declare_dram_parameter("dbg_wt", [128, 4 * 128], F16, isOutput=True)
        dbg_wt1 = nc.declare_dram_parameter("dbg_wt1", [128, 4 * 128], F16, isOutput=True)
        dbg_d2 = nc.declare_dram_parameter("dbg_d2", [128, 4], F32, isOutput=True)
        dbg_sr = nc.declare_dram_parameter("dbg_sr", [128, 4 * C], F32, isOutput=True)
        dbg_s = nc.declare_dram_parameter("dbg_s", [128, 4 * DH], F32, isOutput=True)
        dbg_bd = nc.declare_dram_parameter("dbg_bd", [128, 4 * 128], F16, isOutput=True)

    with tile.TileContext(nc) as tc:
        with (
            tc.tile_pool(name="const", bufs=1) as const,
            tc.tile_pool(name="stash", bufs=1) as stash,
            tc.tile_pool(name="sb", bufs=3) as sb,
            tc.tile_pool(name="xtp", bufs=2) as xtp,
            tc.tile_pool(name="pacc", bufs=1, space="PSUM") as pacc,
            tc.tile_pool(name="pwork", bufs=3, space="PSUM") as pwork,
            tc.tile_pool(name="dram", bufs=1, space="DRAM") as dram,
        ):
            # ---- constants ----
            wst_sb = const.tile([128, 4, E], F16)
            nc.sync.dma_start(wst_sb[:], wst.rearrange("(q p) e -> p q e", p=128))
            w1t_sb = const.tile([128, 4, DH], F32)
            nc.sync.dma_start(w1t_sb[:], w1t.rearrange("(q p) d -> p q d", p=128))
            b1_sb = const.tile([1, DH], F32)
            nc.sync.dma_start(b1_sb[:], b1[:])
            b3_sb = const.tile([1, DH], F32)
            nc.sync.dma_start(b3_sb[:], b3[:])
            small = {}
            for name, t_ in (("wqt", wqt), ("wkt", wkt), ("wvt", wvt), ("w3t", w3t)):
                s_ = const.tile([DH, DH], F32, tag=name, name=name + "_sb")
                nc.sync.dma_start(s_[:], t_[:])
                small[name] = s_
            if with_bslice:
                bse_sb = const.tile([1, E], F16)
                nc.sync.dma_start(bse_sb[:], bse[:])
            tmask_sb = const.tile([128, 1], F32)
            nc.sync.dma_start(tmask_sb[:], tmask[:])
            ones16 = const.tile([128, 1], F16)
            nc.vector.memset(ones16[:], 1.0)
            ones16r = const.tile([1, 128], F16)
            nc.vector.memset(ones16r[:], 1.0)
            ones32r = const.tile([1, 128], F32)
            nc.vector.memset(ones32r[:], 1.0)
            ident = const.tile([128, 128], F32)
            make_identity(nc, ident[:])

            # persistent w^T stash: [e-in-chunk, tile, chunk, n-in-tile] fp16
            wt_stash = stash.tile([128, T, 4, 128], F16)

            # PSUM accumulators (accumulate across all T tiles)
            ps_sr = [pacc.tile([128, C], F32, tag=f"sr{j}", name=f"ps_sr{j}") for j in range(4)]
            ps_d = pacc.tile([1, E], F32, tag="d")

            # ================= Phase A: per-tile streaming =================
            pipe = []
            group_tiles = {}

            def load_group(g):
                r0g = g * 512
                if r0g >= RPC:
                    return
                xt4 = xtp.tile([128, 4, 512], F16, tag="xt4", bufs=3,
                               name=f"xt4_{g}")
                xn4 = xtp.tile([128, 4, C], F16, tag="xn4", bufs=3,
                               name=f"xn4_{g}")
                nrows = min(512, RPC - r0g)
                nc.sync.dma_start(xt4[:, :, :nrows], xh[r0g:r0g + nrows, :],
                                  transpose=True)
                na = nrows // 128
                nc.gpsimd.dma_start(
                    xn4[:, :na, :],
                    xh[r0g:r0g + nrows, :].rearrange("(a p) c -> p a c", p=128),
                )
                group_tiles[g] = (xt4, xn4)

            load_group(0)
            load_group(1)

            def emit_sraw(pt, pww, pxn4):
                for j in range(4):
                    nc.tensor.matmul(ps_sr[j][:], pww[:, ts(j, 128)],
                                     pxn4[:, pt % 4, :],
                                     start=(pt == 0), stop=(pt == T - 1))
                nc.tensor.matmul(ps_d[:], ones16[:], pww[:],
                                 start=(pt == 0), stop=(pt == T - 1))

            for t in range(T):
                r0 = t * 128
                g4 = t % 4
                if g4 == 0:
                    load_group(t // 4 + 2)
                    xt4, xn4 = group_tiles.pop(t // 4) if t // 4 in group_tiles else group_tiles[t // 4]

                # logits: psum [n, e] = sum_q xT_q.T @ wstT_q
                psl = pwork.tile([128, E], F32, tag="pwork")
                for q in range(4):
                    nc.tensor.matmul(
                        psl[:], xt4[:, q, ts(g4, 128)], wst_sb[:, q, :],
                        start=(q == 0), stop=(q == 3 and not with_bslice),
                    )
                if with_bslice:
                    nc.tensor.matmul(psl[:], ones16r[:], bse_sb[:],
                                     start=False, stop=True)

                # u = exp(logits); z = per-head rowsum; w = u / z   (fp16)
                u = sb.tile([128, E], F32, tag="u")
                nc.scalar.activation(u[:], psl[:], mybir.ActivationFunctionType.Exp)
                z = sb.tile([128, H], F32, tag="z")
                nc.vector.reduce_sum(
                    z[:], u[:].rearrange("p (h m) -> p h m", h=H),
                    axis=mybir.AxisListType.X,
                )
                zi = sb.tile([128, H], F32, tag="zi")
                nc.vector.reciprocal(zi[:], z[:])
                if t == T - 1:
                    nc.vector.tensor_scalar_mul(zi[:], zi[:], tmask_sb[:])
                ww = sb.tile([128, E], F16, tag="ww", bufs=5)
                nc.vector.tensor_mul(
                    ww[:].rearrange("p (h m) -> p h m", h=H),
                    u[:].rearrange("p (h m) -> p h m", h=H),
                    zi[:, :, None].broadcast_to([128, H, M]),
                )

                if len(pipe) >= 3:
                    emit_sraw(*pipe.pop(0))
                pipe.append((t, ww, xn4))

                if debug and t == 0:
                    nc.sync.dma_start(dbg_ww[:], ww[:])
                    nc.sync.dma_start(dbg_xt[:], xt4[:].rearrange("p q f -> p (q f)"))
                # stash w^T (single SBUF->SBUF DMA transpose, 3D out)
                nc.sync.dma_start(wt_stash[:, t], ww[:], transpose=True)

            if debug:
                nc.sync.dma_start(dbg_wt[:], wt_stash[:, 0].rearrange("p j n -> p (j n)"))
                nc.sync.dma_start(dbg_wt1[:], wt_stash[:, 1].rearrange("p j n -> p (j n)"))
            for entry in pipe:
                emit_sraw(*entry)
            pipe = []

            # ================= Phase B: AllReduce =================
            sr_sb = sb.tile([128, 4, C], F32, tag="sr_sb", bufs=1)
            for j in range(4):
                nc.scalar.copy(sr_sb[:, j, :], ps_sr[j][:])
            d_sb = sb.tile([1, E], F32, tag="d_sb", bufs=1)
            nc.scalar.copy(d_sb[:], ps_d[:])

            cc_in = dram.tile([129, 2048], F32)
            cc_out = dram.tile([129, 2048], F32, addr_space="Shared")
            nc.sync.dma_start(cc_in[0:128, :], sr_sb[:].rearrange("p j c -> p (j c)"))
            nc.sync.dma_start(cc_in[128:129, 0:E], d_sb[:])
            if single:
                nc.sync.dma_start(cc_out[:], cc_in[:])
            else:
                nc.gpsimd.collective_compute(
                    "AllReduce", mybir.AluOpType.add,
                    replica_groups=[list(range(NCORES))],
                    ins=[cc_in.opt()], outs=[cc_out.opt()],
                )
            sr2 = sb.tile([128, 4, C], F32, tag="sr2", bufs=1)
            nc.sync.dma_start(sr2[:].rearrange("p j c -> p (j c)"), cc_out[0:128, :])
            d2 = sb.tile([128, 4], F32, tag="d2", bufs=1)
            nc.sync.dma_start(d2[:], cc_out[128, 0:E].rearrange("(j p) -> p j", p=128))

            if debug:
                nc.sync.dma_start(dbg_d2[:], d2[:])
                nc.sync.dma_start(dbg_sr[:], sr2[:].rearrange("p j c -> p (j c)"))
            # ================= Phase C: tiny attention (replicated) ========
            dd = sb.tile([128, 4], F32, tag="dd", bufs=1)
            nc.vector.tensor_scalar_add(dd[:], d2[:], 1e-5)
            nc.vector.reciprocal(dd[:], dd[:])
            sn = sb.tile([128, 4, C], F32, tag="sn", bufs=1)
            for j in range(4):
                nc.vector.tensor_scalar_mul(sn[:, j, :], sr2[:, j, :], dd[:, j:j + 1])

            # sn^T: [c-chunk q partitions, e] fp32
            snt = sb.tile([128, 4, E], F32, tag="snt", bufs=1)
            for q in range(4):
                pst = pwork.tile([128, E], F32, tag="pwork")
                for j in range(4):
                    nc.tensor.transpose(pst[:, ts(j, 128)], sn[:, j, ts(q, 128)],
                                        ident[:])
                nc.scalar.copy(snt[:, q, :], pst[:])

            # s[e, dh] = sn @ w1^T + b1
            s_sb = sb.tile([128, 4, DH], F32, tag="s_sb", bufs=1)
            for g in range(4):
                pss = pwork.tile([128, DH], F32, tag="pwork")
                for q in range(4):
                    nc.tensor.matmul(pss[:], snt[:, q, ts(g, 128)], w1t_sb[:, q, :],
                                     start=(q == 0), stop=False)
                nc.tensor.matmul(pss[:], ones32r[:], b1_sb[:], start=False, stop=True)
                nc.scalar.copy(s_sb[:, g, :], pss[:])

            # per-head attention; bd[g] = blockdiag(s_out_{2g}, s_out_{2g+1}) fp16
            bd = [const.tile([128, 128], F16, tag=f"bd{g}", name=f"bd{g}") for g in range(4)]
            for g in range(4):
                nc.vector.memset(bd[g][:], 0.0)
            for h in range(H):
                g, half = h // 2, (h % 2) * 64
                s_h = s_sb[half:half + 64, g, :]
                pt = pwork.tile([64, 64], F32, tag="pwork")
                nc.tensor.transpose(pt[:], s_h, ident[half:half + 64, half:half + 64])
                st_h = sb.tile([64, 64], F32, tag="st_h")
                nc.scalar.copy(st_h[:], pt[:])

                pq = pwork.tile([64, 64], F32, tag="pwork")
                nc.tensor.matmul(pq[:], small["wqt"][:], st_h[:])
                qt_h = sb.tile([64, 64], F32, tag="qt_h")
                nc.scalar.copy(qt_h[:], pq[:])
                pk = pwork.tile([64, 64], F32, tag="pwork")
                nc.tensor.matmul(pk[:], small["wkt"][:], st_h[:])
                kt_h = sb.tile([64, 64], F32, tag="kt_h")
                nc.scalar.copy(kt_h[:], pk[:])
                pv = pwork.tile([64, 64], F32, tag="pwork")
                nc.tensor.matmul(pv[:], st_h[:], small["wvt"][:])
                v_h = sb.tile([64, 64], F32, tag="v_h")
                nc.scalar.copy(v_h[:], pv[:])

                pA = pwork.tile([64, 64], F32, tag="pwork")
                nc.tensor.matmul(pA[:], qt_h[:], kt_h[:])
                mx = sb.tile([64, 1], F32, tag="mx")
                nc.vector.reduce_max(mx[:], pA[:], axis=mybir.AxisListType.X)
                nmx = sb.tile([64, 1], F32, tag="nmx")
                nc.vector.tensor_scalar_mul(nmx[:], mx[:], -1.0)
                p_sb = sb.tile([64, 64], F32, tag="p_sb")
                nc.scalar.activation(p_sb[:], pA[:],
                                     mybir.ActivationFunctionType.Exp, bias=nmx[:])
                rs = sb.tile([64, 1], F32, tag="rs")
                nc.vector.reduce_sum(rs[:], p_sb[:], axis=mybir.AxisListType.X)
                ri = sb.tile([64, 1], F32, tag="ri")
                nc.vector.reciprocal(ri[:], rs[:])
                pn = sb.tile([64, 64], F32, tag="pn")
                nc.vector.tensor_scalar_mul(pn[:], p_sb[:], ri[:])

                pat = pwork.tile([64, 64], F32, tag="pwork")
                nc.tensor.transpose(pat[:], pn[:], ident[0:64, 0:64])
                at_h = sb.tile([64, 64], F32, tag="at_h")
                nc.scalar.copy(at_h[:], pat[:])

                psat = pwork.tile([64, 64], F32, tag="pwork")
                nc.tensor.matmul(psat[:], v_h[:], at_h[:])
                sat_h = sb.tile([64, 64], F32, tag="sat_h")
                nc.scalar.copy(sat_h[:], psat[:])

                pso = pwork.tile([128, 64], F32, tag="pwork")
                nc.tensor.matmul(pso[half:half + 64, :], sat_h[:], small["w3t"][:],
                                 start=True, stop=False)
                nc.tensor.matmul(pso[half:half + 64, :], ones32r[:, 0:64], b3_sb[:],
                                 start=False, stop=True)
                nc.scalar.copy(bd[g][half:half + 64, half:half + 64],
                               pso[half:half + 64, :])

            if debug:
                nc.sync.dma_start(dbg_s[:], s_sb[:].rearrange("p j d -> p (j d)"))
                for g in range(4):
                    nc.sync.dma_start(dbg_bd[:, g * 128:(g + 1) * 128], bd[g][:])
            # ================= Phase D: deslice =================
            for t in range(T):
                g4 = t % 4
                if g4 == 0:
                    osb4 = xtp.tile([128, 4, C], F16, tag="osb4", bufs=4)
                po = pwork.tile([128, C], F32, tag="pwork")
                for g in range(4):
                    nc.tensor.matmul(po[:, ts(g, 128)], wt_stash[:, t, g, :],
                                     bd[g][:], start=True, stop=True)
                if t % 2 == 0:
                    nc.scalar.copy(osb4[:, g4, :], po[:])
                else:
                    nc.vector.tensor_copy(osb4[:, g4, :], po[:])
                if g4 == 3 or t == T - 1:
                    r0g = (t - g4) * 128
                    nrows = min(512, RPC - r0g)
                    na = nrows // 128
                    nc.gpsimd.dma_start(
                        out[r0g:r0g + nrows, :].rearrange("(a p) c -> p a c", p=128),
                        osb4[:, :na, :],
                    )

    nc.finalize()
    return nc


_CACHE = {}


TAIL = TAIL_REAL


def kernel(x, temperature, w_slice, b_slice, w1, b1, wq, wk, wv, w3, b3):
    x = np.asarray(x, np.float32)
    temperature = np.asarray(temperature, np.float32)
    w_slice = np.asarray(w_slice, np.float32)
    b_slice = np.asarray(b_slice, np.float32)
    w1 = np.asarray(w1, np.float32)
    b1 = np.asarray(b1, np.float32)
    wq = np.asarray(wq, np.float32)
    wk = np.asarray(wk, np.float32)
    wv = np.asarray(wv, np.float32)
    w3 = np.asarray(w3, np.float32)
    b3 = np.asarray(b3, np.float32)

    temp = np.clip(temperature.reshape(H), 0.1, 5.0)
    se = np.repeat(1.0 / temp, M)                      # per-e scale
    weff_t = (w_slice * se[:, None]).T                 # [C, E]
    beff = b_slice * se
    with_bslice = bool(np.any(beff != 0.0))

    key = with_bslice
    if key not in _CACHE:
        _CACHE[key] = build(with_bslice)
    nc = _CACHE[key]

    xp = np.zeros((NCORES, RPC, C), np.float16)
    rows = N // NCORES
    xs = x.reshape(N, C)
    for i in range(NCORES):
        xp[i, :rows] = xs[i * rows:(i + 1) * rows]

    base = {
        "wst": weff_t.astype(np.float16),
        "w1t": np.ascontiguousarray(w1.T),
        "b1": b1.reshape(1, DH),
        "wqt": np.ascontiguousarray(wq.T) * np.float32(1.0 / np.sqrt(DH)),
        "wkt": np.ascontiguousarray(wk.T),
        "wvt": np.ascontiguousarray(wv.T),
        "w3t": np.ascontiguousarray(w3.T),
        "b3": b3.reshape(1, DH),
        "tmask": (np.arange(128) < TAIL_REAL).astype(np.float32).reshape(128, 1),
    }
    if with_bslice:
        base["bse"] = beff.reshape(1, E).astype(np.float16)
    in_maps = [{"xh": xp[i], **base} for i in range(NCORES)]

    res = run_bass_kernel_spmd(nc, in_maps, list(range(NCORES)))
    rows = N // NCORES
    full = np.concatenate([res.results[i]["out"][:rows] for i in range(NCORES)], axis=0)
    return full.reshape(B, N, C).astype(np.float32)
